# revision 3
# baseline (speedup 1.0000x reference)
"""Self-contained kernel for nn_BCNpp_46892452938168.

CDCN-style CNN (central-difference convs + pairwise-channel DBO operator).
Input x: (1,3,128,128) f32 + params pytree. Output: (1,32,32) f32.

This implementation runs the network with jax on the attached neuron
devices (single core; the network has batch=1 so data-parallel batch
sharding is degenerate).  All shapes/constants are hardcoded.
"""
import numpy as np
import jax
import jax.numpy as jnp

THETA = 0.7
SIGMA = 1.0
MAP = (32, 32)


def _conv(x, w, pad):
    return jax.lax.conv_general_dilated(
        x, w, (1, 1), ((pad, pad), (pad, pad)),
        dimension_numbers=('NCHW', 'OIHW', 'NCHW'))


def _cdc(x, w):
    out = _conv(x, w, 1)
    kd = w.sum(axis=(2, 3))[:, :, None, None]
    return out - THETA * _conv(x, kd, 0)


def _bn_relu(x, s, b):
    return jax.nn.relu(x * s[None, :, None, None] + b[None, :, None, None])


def _cell(x, layers):
    for (w, s, b) in layers:
        x = _bn_relu(_cdc(x, w), s, b)
    return x


def _maxpool(x):
    return jax.lax.reduce_window(
        x, -jnp.inf, jax.lax.max,
        (1, 1, 3, 3), (1, 1, 2, 2), ((0, 0), (0, 0), (1, 1), (1, 1)))


def _dbo(x, sigma=SIGMA):
    # Tiled over spatial H so the (B,H,W,C,C) pairwise tensor only
    # materializes in chunks (flash-style blocking over HW).
    b, c, h, w = x.shape
    xp = jnp.transpose(x, (0, 2, 3, 1))            # (B,H,W,C)
    chunks = []
    hc = max(1, min(h, (1 << 22) // (w * c * c)))  # ~4M elems per chunk
    for h0 in range(0, h, hc):
        xq = xp[:, h0:h0 + hc]
        d = xq[..., :, None] - xq[..., None, :]
        g = jnp.exp(-(d * d) / (sigma * sigma))
        num = jnp.einsum('bhwij,bhwi->bhwj', g, xq)
        den = jnp.sum(g, axis=-2)
        chunks.append(num / den)
    out = jnp.concatenate(chunks, axis=1) if len(chunks) > 1 else chunks[0]
    return jnp.transpose(out, (0, 3, 1, 2))


def _sa(x, w):
    avg = jnp.mean(x, axis=1, keepdims=True)
    mx = jnp.max(x, axis=1, keepdims=True)
    a = jnp.concatenate([avg, mx], axis=1)
    k = w.shape[-1]
    return jax.nn.sigmoid(_conv(a, w, k // 2))


def _resize32(x):
    b, c = x.shape[0], x.shape[1]
    return jax.image.resize(x, (b, c, MAP[0], MAP[1]), method='bilinear')


def _forward(x, params):
    w, s, b = params['conv1']
    x = _bn_relu(_cdc(x, w), s, b)

    xr1 = _cell(x, params['b1'])
    xb1 = _cell(_dbo(x), params['b1d'])
    blk1 = _maxpool(xr1 + xb1)
    xr1 = _maxpool(xr1)
    blk1_32 = _resize32(_sa(blk1, params['sa1']) * blk1)

    xr2 = _cell(xr1, params['b2'])
    xb2 = _cell(_dbo(xr1), params['b2d'])
    blk2 = _maxpool(xr2 + xb2)
    xr2 = _maxpool(xr2)
    blk2_32 = _resize32(_sa(blk2, params['sa2']) * blk2)

    xr3 = _cell(xr2, params['b3'])
    xb3 = _cell(_dbo(xr2), params['b3d'])
    blk3 = _maxpool(xr3 + xb3)
    blk3_32 = _resize32(_sa(blk3, params['sa3']) * blk3)

    xc = jnp.concatenate([blk1_32, blk2_32, blk3_32], axis=1)
    w, s, b = params['last1']
    h = _bn_relu(_cdc(xc, w), s, b)
    m = jax.nn.relu(_cdc(h, params['last2']))
    return m[:, 0]


def kernel(**inputs) -> np.ndarray:
    x = inputs['x']
    params = inputs['params']
    out = _forward(jnp.asarray(x), jax.tree_util.tree_map(jnp.asarray, params))
    return np.asarray(out, dtype=np.float32)
